# revision 16
# baseline (speedup 1.0000x reference)
"""Trainium2 Bass kernel for nn_MultiHeadAttention_446676599023.

Strategy (8 NeuronCores, SPMD, no collectives):
  core c -> batch b = c//2, head-group g = c%2 (heads 8g..8g+7, E-dims 512g..512g+512).

Math: reference computes attn_out = softmax(QK^T/sqrt(D)) @ V per head, projects with
Wo, takes mean over sequence, normalizes, subtracts text_array, then a tiny MLP.
mean_S commutes with the output projection, so each core only needs
  r_h[d] = sum_q softmax_row(q) @ V_h  summed over q   (shape [64] per head)
and the whole Wo/normalize/MLP tail runs on host on a [4,1024] tensor (exact algebra,
negligible FLOPs). Device work per core:
  - x^T and all weights live in SBUF as bf16 (halves DMA, enables fast weight load).
  - Q^T,K^T = (Wq x^T) in [d-part, seq-free] bf16; V = x Wv^T in [seq-part, d-free] bf16
    with a ones column per head (65-stride) so the E@V matmul also emits the softmax
    denominator Z as row 64.
  - scores^T[k,q]: lhsT=K^T slice, rhs=Q^T slice (contraction d=64; even/odd heads on
    PE row-groups 0/64 run concurrently).
  - E = exp(scores/8 + maskbias_k), split across two engines to balance throughput:
    ScalarE table exp for 11/16 key-tiles, DVE Schraudolph exp (int32 affine + bitcast,
    ~2-4% rel err, attenuated to <1e-3 by the normalize/subtract tail) for 5/16.
  - P^T[d,q] (+ Z row) = matmul(lhsT=V_aug[k,65], rhs=E^T[k,q]) accumulated over k.
  - finalize per q-chunk: one DVE copy drains P^T/Z to SBUF bf16, DMA ships it out.
Host does r[d] = sum_q P[d,q]/Z[q] (0.014%% of total FLOPs), then /S and the exact
Wo/normalize/MLP tail.
"""

import math
import os
import sys

import numpy as np

for _p in ("/opt/trn_rl_repo",):
    if _p not in sys.path and os.path.isdir(_p):
        sys.path.append(_p)

B, S, E, H = 4, 2048, 1024, 16
D = E // H            # 64 head dim
G = 2                 # head groups (tensor-parallel factor)
EG = E // G           # 512 dims per group
HG = H // G           # 8 heads per group
NCORES = 8
PART = 128
ET = E // PART        # 8 contraction tiles for projections
KT = S // PART        # 16 key tiles
MT = EG // PART       # 4 m-tiles (= head pairs) per group
NEG = -1.0e30

# Schraudolph fast-exp constants in bf16-bit space: bitcast(int16(x*EXA + EXB))
# ~= exp(x) as bf16 (the HW matmul rejects mixed 32/16-bit inputs, so the DVE
# exp path must emit bf16 to match the bf16 V operand).
EXA = 184.6649652337873  # 2^7 / ln(2)
EXB = 16250.409          # 127 * 2^7 - 366393/65536 (min max-rel-err bias)
DVE_KT = frozenset((2, 4, 7, 9, 12, 14))  # key-tiles whose exp runs on DVE

_CACHE: dict = {}


def _build(repeat: int = 1):
    """Build the Bacc module (one SPMD program, same on all 8 cores)."""
    ablate = set(os.environ.get("BASS_V2_ABL", "").split(","))
    import concourse.bacc as bacc
    import concourse.mybir as mybir
    import concourse.tile as tile
    from contextlib import ExitStack

    f32 = mybir.dt.float32
    f32r = mybir.dt.float32r
    bf16 = mybir.dt.bfloat16
    i16 = mybir.dt.int16
    AF = mybir.ActivationFunctionType
    AX = mybir.AxisListType
    ALU = mybir.AluOpType

    nc = bacc.Bacc("TRN2", target_bir_lowering=False, debug=False)
    xT = nc.dram_tensor("xT", [E, S], bf16, kind="ExternalInput").ap()
    wqT = nc.dram_tensor("wqT", [E, EG], bf16, kind="ExternalInput").ap()
    wkT = nc.dram_tensor("wkT", [E, EG], bf16, kind="ExternalInput").ap()
    wvT = nc.dram_tensor("wvT", [E, EG], bf16, kind="ExternalInput").ap()
    mbT = nc.dram_tensor("mbT", [PART, KT], f32, kind="ExternalInput").ap()
    mb2T = nc.dram_tensor("mb2T", [PART, KT], f32, kind="ExternalInput").ap()
    bqT = nc.dram_tensor("bqT", [PART, MT], f32, kind="ExternalInput").ap()
    bkT = nc.dram_tensor("bkT", [PART, MT], f32, kind="ExternalInput").ap()
    resP = nc.dram_tensor(
        "resP", [repeat, HG, 65, S], bf16, kind="ExternalOutput"
    ).ap()

    QC = 4          # q chunks of 512
    QW = S // QC    # 512

    with tile.TileContext(nc) as tc, ExitStack() as ctx:
        const_p = ctx.enter_context(tc.tile_pool(name="const", bufs=1))
        xt_p = ctx.enter_context(tc.tile_pool(name="xt", bufs=ET))
        wv_p = ctx.enter_context(tc.tile_pool(name="wv", bufs=ET))
        wqk_p = ctx.enter_context(tc.tile_pool(name="wqk", bufs=24))
        qt_p = ctx.enter_context(tc.tile_pool(name="qt", bufs=2))
        kt_p = ctx.enter_context(tc.tile_pool(name="kt", bufs=2))
        v_p = ctx.enter_context(tc.tile_pool(name="v", bufs=KT))
        et_p = ctx.enter_context(tc.tile_pool(name="et", bufs=6))
        ei_p = ctx.enter_context(tc.tile_pool(name="ei", bufs=6))
        psb_p = ctx.enter_context(tc.tile_pool(name="psb", bufs=4))
        # PSUM budget (8 banks): scores 5x[128,512] = 5 (per-head tiles so the
        # two exp engines run concurrently on different heads), pacc 2x[65,512]
        # = 2, projection accumulator 1x[128,512] = 1.
        sc_ps = ctx.enter_context(tc.tile_pool(name="scps", bufs=5, space="PSUM"))
        p_ps = ctx.enter_context(tc.tile_pool(name="pps", bufs=2, space="PSUM"))
        qk_ps = ctx.enter_context(tc.tile_pool(name="qkps", bufs=1, space="PSUM"))

        for rep in range(repeat):
            mb = const_p.tile([PART, KT], f32, tag="mb")
            nc.sync.dma_start(mb[:], mbT[:])
            mb2 = const_p.tile([PART, KT], f32, tag="mb2")
            nc.sync.dma_start(mb2[:], mb2T[:])
            bq = const_p.tile([PART, MT], f32, tag="bq")
            nc.sync.dma_start(bq[:], bqT[:])
            bk = const_p.tile([PART, MT], f32, tag="bk")
            nc.sync.dma_start(bk[:], bkT[:])
            ones_c = const_p.tile([PART, HG], bf16, tag="ones_c")
            nc.vector.memset(ones_c[:], 1.0)


            # ---- emission helpers (program order on each engine queue matters:
            # the PE executes in order, so Q/K projection and V projection are
            # interleaved into the attention stream to fill exp-wait gaps) ----
            v_sb = [None] * KT

            def emit_vproj(ks):
                vt = v_p.tile([PART, HG * 65], bf16, tag="v")
                v3 = vt[:].rearrange("p (h c) -> p h c", c=65)
                nc.vector.tensor_copy(
                    v3[:, :, 64:65], ones_c[:].rearrange("p (h o) -> p h o", o=1)
                )
                ps = qk_ps.tile([PART, EG], f32, tag="qkp", name="vps")
                for i in range(ET):
                    nc.tensor.matmul(
                        ps[:],
                        lhsT=xt[i][:, ks * PART : (ks + 1) * PART],
                        rhs=wv[i][:],
                        start=(i == 0),
                        stop=(i == ET - 1),
                    )
                nc.vector.tensor_copy(
                    v3[:, :, 0:64],
                    ps[:].rearrange("p (h c) -> p h c", c=64),
                )
                v_sb[ks] = vt

            qkmats = {}
            wdma = {}

            def emit_wqk_dma(p):
                tiles = []
                for wT in (wkT, wqT):
                    for i in range(ET):
                        t = wqk_p.tile([PART, PART], bf16, tag="wqk")
                        nc.sync.dma_start(
                            t[:],
                            wT[i * PART : (i + 1) * PART, p * PART : (p + 1) * PART],
                        )
                        tiles.append(t)
                wdma[p] = tiles

            def emit_qkproj_group(p, j):
                # j in 0..7: j//4 selects K (0) / Q (1), j%4 the q-chunk
                if j == 0:
                    qkmats[p] = (
                        kt_p.tile([PART, S], bf16, tag="kt", name=f"kt{p}"),
                        qt_p.tile([PART, S], bf16, tag="qt", name=f"qt{p}"),
                    )
                which, qc2 = j // 4, j % 4
                dst = qkmats[p][which]
                bias = (bk, bq)[which]
                wtiles = wdma[p][which * ET : (which + 1) * ET]
                ps = qk_ps.tile([PART, QW], f32, tag="qkp", name="qkps")
                for i in range(ET):
                    nc.tensor.matmul(
                        ps[:],
                        lhsT=wtiles[i][:],
                        rhs=xt[i][:, qc2 * QW : (qc2 + 1) * QW],
                        start=(i == 0),
                        stop=(i == ET - 1),
                    )
                nc.vector.tensor_scalar_add(
                    dst[:, qc2 * QW : (qc2 + 1) * QW],
                    ps[:],
                    bias[:, p : p + 1],
                )

            # DMA order: small weight tiles first (they gate the first
            # projection groups), then x spread across four engine DGE queues
            # so the 4MB load parallelizes across DMA engines
            emit_wqk_dma(0)
            wv = []
            for i in range(ET):
                t = wv_p.tile([PART, EG], bf16, tag="wv")
                nc.sync.dma_start(t[:], wvT[i * PART : (i + 1) * PART, :])
                wv.append(t)
            xt = []
            for i in range(ET):
                t = xt_p.tile([PART, S], bf16, tag="xt")
                nc.sync.dma_start(t[:], xT[i * PART : (i + 1) * PART, :])
                xt.append(t)
            # ---- prologue: Q/K projection for head-pair 0 interleaved with
            # most of the V projection (all pure PE+DVE work that overlaps the
            # input DMA; psum ping-pong keeps the PE streaming) ----
            for j in range(8):
                emit_qkproj_group(0, j)
                emit_vproj(2 * j)

            # ---- attention over head-pairs; V proj folds into (p0, qc0) and
            # next head-pair's Q/K proj into the remaining q-chunks ----
            for p in range(MT):
                kt_m, qt_m = qkmats.pop(p)
                if p + 1 < MT:
                    emit_wqk_dma(p + 1)
                for qc in range(QC):
                    # filler PE work interleaved into the kt loop (executes
                    # during exp waits): V projection streams through (p0,qc0)
                    # two tiles ahead of its consumer; the next head-pair's Q/K
                    # projection spreads over the remaining q-chunks
                    vproj_pipe = p == 0 and qc == 0
                    if p == 0:
                        spread = {0: [], 1: [0, 1, 2], 2: [3, 4, 5], 3: [6, 7]}
                        fillers = [
                            (emit_qkproj_group, 1, j) for j in spread[qc]
                        ]
                    elif p + 1 < MT:
                        fillers = [
                            (emit_qkproj_group, p + 1, j)
                            for j in range(qc * 2, qc * 2 + 2)
                        ]
                    else:
                        fillers = []
                    pacc = {}
                    for hl in (0, 1):
                        pacc[hl] = p_ps.tile(
                            [65, QW], f32, tag="pp", name=f"pacc{hl}"
                        )
                    sct = [[None, None] for _ in range(KT)]

                    def emit_scores(kt):
                        for hl in (0, 1):
                            scps = sc_ps.tile([PART, QW], f32, tag="sc")
                            r0 = hl * 64
                            nc.tensor.matmul(
                                scps[:],
                                lhsT=kt_m[r0 : r0 + 64, kt * PART : (kt + 1) * PART],
                                rhs=qt_m[r0 : r0 + 64, qc * QW : (qc + 1) * QW],
                            )
                            sct[kt][hl] = scps

                    emit_scores(0)
                    for kt in range(KT):
                        # software pipeline: next kt's scores go to the PE queue
                        # BEFORE this kt's EV so the (in-order) PE never blocks
                        # on the exp result with ready work behind it
                        if kt + 1 < KT:
                            emit_scores(kt + 1)
                        if vproj_pipe and kt % 2 == 0:
                            emit_vproj(2 * (kt // 2) + 1)
                        if fillers and kt in (3, 8, 13):
                            f = fillers.pop(0)
                            f[0](*f[1:])
                        e_ap = [None, None]
                        # head A: ScalarE table exp; head B: DVE Schraudolph --
                        # the two engines work the same kt concurrently
                        e = et_p.tile([PART, QW], bf16, tag="et")
                        nc.scalar.activation(
                            e[:],
                            sct[kt][0][:],
                            AF.Exp,
                            bias=mb[:, kt : kt + 1],
                            scale=1.0 / math.sqrt(D),
                        )
                        e_ap[0] = e[:]
                        ei = ei_p.tile([PART, QW], i16, tag="ei")
                        nc.vector.tensor_scalar(
                            ei[:],
                            sct[kt][1][:],
                            EXA / 8.0,
                            mb2[:, kt : kt + 1],
                            ALU.mult,
                            ALU.add,
                        )
                        e_ap[1] = ei[:].bitcast(bf16)
                        for hl in (0, 1):
                            nc.tensor.matmul(
                                pacc[hl][:],
                                lhsT=v_sb[kt][:, 65 * (2 * p + hl) : 65 * (2 * p + hl) + 65],
                                rhs=e_ap[hl],
                                start=(kt == 0),
                                stop=(kt == KT - 1),
                            )
                    # drain P^T/Z to SBUF bf16 and ship to host, which does the
                    # per-q invZ scaling + reduction (tiny)
                    for hl in (0, 1):
                        psb = psb_p.tile([65, QW], bf16, tag="psb")
                        if hl == 0:
                            nc.scalar.copy(psb[:], pacc[hl][:])
                        else:
                            nc.vector.tensor_copy(psb[:], pacc[hl][:])
                        nc.sync.dma_start(
                            resP[rep, 2 * p + hl, :, qc * QW : (qc + 1) * QW],
                            psb[:],
                        )

    nc.compile()
    return nc


def get_nc(repeat: int = 1):
    key = ("nc", repeat, os.environ.get("BASS_V2_ABL", ""))
    if key not in _CACHE:
        _CACHE[key] = _build(repeat)
    return _CACHE[key]


def make_in_maps(x, mask, Wq, bq, Wk, bk, Wv):
    """Per-core input dict (core c -> batch c//2, head-group c%2)."""
    import ml_dtypes

    bf = ml_dtypes.bfloat16
    x = np.asarray(x, np.float32)
    mask = np.asarray(mask)
    maskbias = (mask == 0).astype(np.float32) * NEG  # [B, S]
    mb2 = np.clip(
        maskbias.astype(np.float64) * EXA + EXB, -3.0e38, 3.0e38
    ).astype(np.float32)  # masked rows saturate int16 -> 0x8000 -> bf16 -0.0
    in_maps = []
    xTb = [np.ascontiguousarray(x[b].T.astype(bf)) for b in range(B)]
    mbTb = [np.ascontiguousarray(maskbias[b].reshape(KT, PART).T) for b in range(B)]
    mb2Tb = [np.ascontiguousarray(mb2[b].reshape(KT, PART).T) for b in range(B)]
    slabs = {}
    for g in range(G):
        sl = slice(g * EG, (g + 1) * EG)
        slabs[g] = (
            np.ascontiguousarray(np.asarray(Wq, np.float32)[sl].T.astype(bf)),
            np.ascontiguousarray(np.asarray(Wk, np.float32)[sl].T.astype(bf)),
            np.ascontiguousarray(np.asarray(Wv, np.float32)[sl].T.astype(bf)),
            np.ascontiguousarray(np.asarray(bq, np.float32)[sl].reshape(MT, PART).T),
            np.ascontiguousarray(np.asarray(bk, np.float32)[sl].reshape(MT, PART).T),
        )
    for c in range(NCORES):
        b, g = c // G, c % G
        wq_t, wk_t, wv_t, bq_t, bk_t = slabs[g]
        in_maps.append(
            {
                "xT": xTb[b],
                "wqT": wq_t,
                "wkT": wk_t,
                "wvT": wv_t,
                "mbT": mbTb[b],
                "mb2T": mb2Tb[b],
                "bqT": bq_t,
                "bkT": bk_t,
            }
        )
    return in_maps


def host_tail(mean_attn, text_array, bv, Wo, bo, W1, b1, W2, b2):
    """Exact tail on [B, E]: out_proj (after the mean), normalize, sub, MLP."""
    out = mean_attn + np.asarray(bv, np.float32)[None, :]
    out = out @ np.asarray(Wo, np.float32).T + np.asarray(bo, np.float32)
    out = out / np.linalg.norm(out, axis=-1, keepdims=True)
    out = out - np.asarray(text_array, np.float32)
    h = np.maximum(out @ np.asarray(W1, np.float32).T + np.asarray(b1, np.float32), 0.0)
    return np.tanh(h @ np.asarray(W2, np.float32).T + np.asarray(b2, np.float32))


def kernel(
    x, mask, text_array, Wq, bq, Wk, bk, Wv, bv, Wo, bo, W1, b1, W2, b2
):
    from concourse.bass_utils import run_bass_kernel_spmd

    nc = get_nc()
    in_maps = make_in_maps(x, mask, Wq, bq, Wk, bk, Wv)
    out = run_bass_kernel_spmd(nc, in_maps, core_ids=list(range(NCORES)))
    mean_attn = np.zeros((B, E), np.float32)
    for c in range(NCORES):
        b, g = c // G, c % G
        pz = np.asarray(out.results[c]["resP"][0], np.float32)  # [HG, 65, S]
        r = np.einsum("hdq,hq->hd", pz[:, 0:64, :], 1.0 / pz[:, 64, :])
        mean_attn[b, g * EG : (g + 1) * EG] = r.reshape(EG) / S
    return host_tail(mean_attn, text_array, bv, Wo, bo, W1, b1, W2, b2).astype(
        np.float32
    )


# revision 21
# speedup vs baseline: 1.0215x; 1.0215x over previous
"""Trainium2 Bass kernel for nn_MultiHeadAttention_446676599023.

Strategy (8 NeuronCores, SPMD, no collectives):
  core c -> batch b = c//2, head-group g = c%2 (heads 8g..8g+7, E-dims 512g..512g+512).

Math: reference computes attn_out = softmax(QK^T/sqrt(D)) @ V per head, projects with
Wo, takes mean over sequence, normalizes, subtracts text_array, then a tiny MLP.
mean_S commutes with the output projection, so each core only needs
  r_h[d] = sum_q softmax_row(q) @ V_h  summed over q   (shape [64] per head)
and the whole Wo/normalize/MLP tail runs on host on a [4,1024] tensor (exact algebra,
negligible FLOPs). Device work per core:
  - x^T and all weights live in SBUF as bf16 (halves DMA, enables fast weight load).
  - Q^T,K^T = (Wq x^T) in [d-part, seq-free] bf16; V = x Wv^T in [seq-part, d-free] bf16
    with a ones column per head (65-stride) so the E@V matmul also emits the softmax
    denominator Z as row 64.
  - scores^T[k,q]: lhsT=K^T slice, rhs=Q^T slice (contraction d=64; even/odd heads on
    PE row-groups 0/64 run concurrently).
  - E = exp(scores/8 + maskbias_k), split across two engines to balance throughput:
    ScalarE table exp for 11/16 key-tiles, DVE Schraudolph exp (int32 affine + bitcast,
    ~2-4% rel err, attenuated to <1e-3 by the normalize/subtract tail) for 5/16.
  - P^T[d,q] (+ Z row) = matmul(lhsT=V_aug[k,65], rhs=E^T[k,q]) accumulated over k.
  - finalize per q-chunk: one DVE copy drains P^T/Z to SBUF bf16, DMA ships it out.
Host does r[d] = sum_q P[d,q]/Z[q] (0.014%% of total FLOPs), then /S and the exact
Wo/normalize/MLP tail.
"""

import math
import os
import sys

import numpy as np

for _p in ("/opt/trn_rl_repo",):
    if _p not in sys.path and os.path.isdir(_p):
        sys.path.append(_p)

B, S, E, H = 4, 2048, 1024, 16
D = E // H            # 64 head dim
G = 2                 # head groups (tensor-parallel factor)
EG = E // G           # 512 dims per group
HG = H // G           # 8 heads per group
NCORES = 8
PART = 128
ET = E // PART        # 8 contraction tiles for projections
KT = S // PART        # 16 key tiles
MT = EG // PART       # 4 m-tiles (= head pairs) per group
NEG = -1.0e30

# Schraudolph fast-exp constants in bf16-bit space: bitcast(int16(x*EXA + EXB))
# ~= exp(x) as bf16 (the HW matmul rejects mixed 32/16-bit inputs, so the DVE
# exp path must emit bf16 to match the bf16 V operand).
EXA = 184.6649652337873  # 2^7 / ln(2)
EXB = 16250.409          # 127 * 2^7 - 366393/65536 (min max-rel-err bias)
DVE_KT = frozenset((2, 4, 7, 9, 12, 14))  # key-tiles whose exp runs on DVE

_CACHE: dict = {}


def _build(repeat: int = 1):
    """Build the Bacc module (one SPMD program, same on all 8 cores)."""
    ablate = set(os.environ.get("BASS_V2_ABL", "").split(","))
    import concourse.bacc as bacc
    import concourse.mybir as mybir
    import concourse.tile as tile
    from contextlib import ExitStack

    f32 = mybir.dt.float32
    bf16 = mybir.dt.bfloat16
    f8 = mybir.dt.float8e4   # V tiles (values ~N(0,1): e4m3 3.4% rms)
    f8e = mybir.dt.float8e5  # e tiles (exp up to e^9.8 needs e5m2 range)
    i16 = mybir.dt.int16
    PM = mybir.MatmulPerfMode
    AF = mybir.ActivationFunctionType
    AX = mybir.AxisListType
    ALU = mybir.AluOpType

    nc = bacc.Bacc("TRN2", target_bir_lowering=False, debug=False)
    xT = nc.dram_tensor("xT", [E, S], bf16, kind="ExternalInput").ap()
    wqT = nc.dram_tensor("wqT", [E, EG], bf16, kind="ExternalInput").ap()
    wkT = nc.dram_tensor("wkT", [E, EG], bf16, kind="ExternalInput").ap()
    wvT = nc.dram_tensor("wvT", [E, EG], bf16, kind="ExternalInput").ap()
    mbT = nc.dram_tensor("mbT", [PART, KT], f32, kind="ExternalInput").ap()
    mbAT = nc.dram_tensor("mbAT", [PART, KT], f32, kind="ExternalInput").ap()
    mb2T = nc.dram_tensor("mb2T", [PART, KT], f32, kind="ExternalInput").ap()
    bqT = nc.dram_tensor("bqT", [PART, MT], f32, kind="ExternalInput").ap()
    bkT = nc.dram_tensor("bkT", [PART, MT], f32, kind="ExternalInput").ap()
    resP = nc.dram_tensor(
        "resP", [repeat, HG, 65, S], bf16, kind="ExternalOutput"
    ).ap()

    QC = 4          # q chunks of 512
    QW = S // QC    # 512

    with tile.TileContext(nc) as tc, ExitStack() as ctx:
        const_p = ctx.enter_context(tc.tile_pool(name="const", bufs=1))
        xt_p = ctx.enter_context(tc.tile_pool(name="xt", bufs=ET))
        wv_p = ctx.enter_context(tc.tile_pool(name="wv", bufs=ET))
        wqk_p = ctx.enter_context(tc.tile_pool(name="wqk", bufs=24))
        qt_p = ctx.enter_context(tc.tile_pool(name="qt", bufs=2))
        kt_p = ctx.enter_context(tc.tile_pool(name="kt", bufs=2))
        v_p = ctx.enter_context(tc.tile_pool(name="v", bufs=KT))
        et_p = ctx.enter_context(tc.tile_pool(name="et", bufs=4))
        ei_p = ctx.enter_context(tc.tile_pool(name="ei", bufs=6))
        psb_p = ctx.enter_context(tc.tile_pool(name="psb", bufs=4))
        # PSUM budget (8 banks): scores 5x[128,512] = 5 (per-head tiles so the
        # two exp engines run concurrently on different heads), pacc 2x[65,512]
        # = 2, projection accumulator 1x[128,512] = 1.
        sc_ps = ctx.enter_context(tc.tile_pool(name="scps", bufs=5, space="PSUM"))
        p_ps = ctx.enter_context(tc.tile_pool(name="pps", bufs=2, space="PSUM"))
        qk_ps = ctx.enter_context(tc.tile_pool(name="qkps", bufs=1, space="PSUM"))

        for rep in range(repeat):
            mb = const_p.tile([PART, KT], f32, tag="mb")
            nc.sync.dma_start(mb[:], mbT[:])
            mbA = const_p.tile([PART, KT], f32, tag="mbA")
            nc.sync.dma_start(mbA[:], mbAT[:])
            mb2 = const_p.tile([PART, KT], f32, tag="mb2")
            nc.sync.dma_start(mb2[:], mb2T[:])
            bq = const_p.tile([PART, MT], f32, tag="bq")
            nc.sync.dma_start(bq[:], bqT[:])
            bk = const_p.tile([PART, MT], f32, tag="bk")
            nc.sync.dma_start(bk[:], bkT[:])
            ones_c = const_p.tile([PART, HG], bf16, tag="ones_c")
            nc.vector.memset(ones_c[:], 1.0)


            # ---- emission helpers (program order on each engine queue matters:
            # the PE executes in order, so Q/K projection and V projection are
            # interleaved into the attention stream to fill exp-wait gaps) ----
            v_sb = [None] * (KT // 2)  # fp8 pair tiles [128, 2, HG*66]

            def emit_vproj(ks):
                j, sub = ks // 2, ks % 2
                if sub == 0:
                    v_sb[j] = v_p.tile([PART, 2, HG * 66], f8, tag="v", name=f"vp{j}")
                v3 = v_sb[j][:][:, sub, :].rearrange("p (h c) -> p h c", c=66)
                nc.vector.memset(v3[:, :, 64:66], 1.0)
                ps = qk_ps.tile([PART, EG], f32, tag="qkp", name="vps")
                for i in range(ET):
                    nc.tensor.matmul(
                        ps[:],
                        lhsT=xt[i][:, ks * PART : (ks + 1) * PART],
                        rhs=wv[i][:],
                        start=(i == 0),
                        stop=(i == ET - 1),
                    )
                # DVE cannot narrow f32->fp8; ScalarE can
                nc.scalar.copy(
                    v3[:, :, 0:64],
                    ps[:].rearrange("p (h c) -> p h c", c=64),
                )

            qkmats = {}
            wdma = {}

            def emit_wqk_dma(p):
                tiles = []
                for wT in (wkT, wqT):
                    for i in range(ET):
                        t = wqk_p.tile([PART, PART], bf16, tag="wqk")
                        nc.sync.dma_start(
                            t[:],
                            wT[i * PART : (i + 1) * PART, p * PART : (p + 1) * PART],
                        )
                        tiles.append(t)
                wdma[p] = tiles

            def emit_qkproj_group(p, j):
                # j in 0..7: j//4 selects K (0) / Q (1), j%4 the q-chunk
                if j == 0:
                    qkmats[p] = (
                        kt_p.tile([PART, S], bf16, tag="kt", name=f"kt{p}"),
                        qt_p.tile([PART, S], bf16, tag="qt", name=f"qt{p}"),
                    )
                which, qc2 = j // 4, j % 4
                dst = qkmats[p][which]
                bias = (bk, bq)[which]
                wtiles = wdma[p][which * ET : (which + 1) * ET]
                ps = qk_ps.tile([PART, QW], f32, tag="qkp", name="qkps")
                for i in range(ET):
                    nc.tensor.matmul(
                        ps[:],
                        lhsT=wtiles[i][:],
                        rhs=xt[i][:, qc2 * QW : (qc2 + 1) * QW],
                        start=(i == 0),
                        stop=(i == ET - 1),
                    )
                nc.vector.tensor_scalar_add(
                    dst[:, qc2 * QW : (qc2 + 1) * QW],
                    ps[:],
                    bias[:, p : p + 1],
                )

            # DMA order: small weight tiles first (they gate the first
            # projection groups), then x spread across four engine DGE queues
            # so the 4MB load parallelizes across DMA engines
            emit_wqk_dma(0)
            wv = []
            for i in range(ET):
                t = wv_p.tile([PART, EG], bf16, tag="wv")
                nc.sync.dma_start(t[:], wvT[i * PART : (i + 1) * PART, :])
                wv.append(t)
            xt = []
            for i in range(ET):
                t = xt_p.tile([PART, S], bf16, tag="xt")
                nc.sync.dma_start(t[:], xT[i * PART : (i + 1) * PART, :])
                xt.append(t)
            # ---- prologue: Q/K projection for head-pair 0 interleaved with
            # most of the V projection (all pure PE+DVE work that overlaps the
            # input DMA; psum ping-pong keeps the PE streaming) ----
            for j in range(8):
                emit_qkproj_group(0, j)
                emit_vproj(2 * j)

            # ---- attention over head-pairs; V proj folds into (p0, qc0) and
            # next head-pair's Q/K proj into the remaining q-chunks ----
            for p in range(MT):
                kt_m, qt_m = qkmats.pop(p)
                if p + 1 < MT:
                    emit_wqk_dma(p + 1)
                for qc in range(QC):
                    # filler PE work interleaved into the kt loop (executes
                    # during exp waits): V projection streams through (p0,qc0)
                    # two tiles ahead of its consumer; the next head-pair's Q/K
                    # projection spreads over the remaining q-chunks
                    vproj_pipe = p == 0 and qc == 0
                    if p == 0:
                        spread = {0: [], 1: [0, 1, 2], 2: [3, 4, 5], 3: [6, 7]}
                        fillers = [
                            (emit_qkproj_group, 1, j) for j in spread[qc]
                        ]
                    elif p + 1 < MT:
                        fillers = [
                            (emit_qkproj_group, p + 1, j)
                            for j in range(qc * 2, qc * 2 + 2)
                        ]
                    else:
                        fillers = []
                    pacc = {}
                    for hl in (0, 1):
                        pacc[hl] = p_ps.tile(
                            [65, QW], f32, tag="pp", name=f"pacc{hl}"
                        )
                    sct = [[None, None] for _ in range(KT)]

                    def emit_scores(kt):
                        for hl in (0, 1):
                            scps = sc_ps.tile([PART, QW], f32, tag="sc")
                            r0 = hl * 64
                            nc.tensor.matmul(
                                scps[:],
                                lhsT=kt_m[r0 : r0 + 64, kt * PART : (kt + 1) * PART],
                                rhs=qt_m[r0 : r0 + 64, qc * QW : (qc + 1) * QW],
                            )
                            sct[kt][hl] = scps

                    emit_scores(0)
                    for kt in range(KT):
                        # software pipeline: next kt's scores go to the PE queue
                        # BEFORE this kt's EV so the (in-order) PE never blocks
                        # on the exp result with ready work behind it
                        if kt + 1 < KT:
                            emit_scores(kt + 1)
                        if vproj_pipe and kt % 2 == 0:
                            emit_vproj(2 * (kt // 2) + 1)
                        if fillers and kt in (3, 8, 13):
                            f = fillers.pop(0)
                            f[0](*f[1:])
                        # head A: ScalarE exp -> fp8 pair tile (feeds a
                        # DoubleRow EV every second kt); head B: DVE Schraudolph
                        # bf16 -- the two engines work the same kt concurrently
                        if kt % 2 == 0:
                            ea_pair = et_p.tile([PART, 2, QW], f8e, tag="et")
                        nc.scalar.activation(
                            ea_pair[:][:, kt % 2, :],
                            sct[kt][0][:],
                            AF.Exp,
                            bias=mb[:, kt : kt + 1],
                            scale=1.0 / math.sqrt(D),
                        )
                        ei = ei_p.tile([PART, QW], i16, tag="ei")
                        nc.vector.tensor_scalar(
                            ei[:],
                            sct[kt][1][:],
                            EXA / 8.0,
                            mb2[:, kt : kt + 1],
                            ALU.mult,
                            ALU.add,
                        )
                        hA = 66 * (2 * p)
                        hB = 66 * (2 * p + 1)
                        if kt % 2 == 1:
                            nc.tensor.matmul(
                                pacc[0][:],
                                lhsT=v_sb[kt // 2][:][:, 0:2, hA : hA + 65],
                                rhs=ea_pair[:][:, 0:2, :],
                                start=(kt == 1),
                                stop=(kt == KT - 1),
                                perf_mode=PM.DoubleRow,
                            )
                        nc.tensor.matmul(
                            pacc[1][:],
                            lhsT=v_sb[kt // 2][:][:, kt % 2, hB : hB + 65],
                            rhs=ei[:].bitcast(bf16),
                            start=(kt == 0),
                            stop=(kt == KT - 1),
                        )
                    # drain P^T/Z to SBUF bf16 and ship to host, which does the
                    # per-q invZ scaling + reduction (tiny)
                    for hl in (0, 1):
                        psb = psb_p.tile([65, QW], bf16, tag="psb")
                        if hl == 0:
                            nc.scalar.copy(psb[:], pacc[hl][:])
                        else:
                            nc.vector.tensor_copy(psb[:], pacc[hl][:])
                        nc.sync.dma_start(
                            resP[rep, 2 * p + hl, :, qc * QW : (qc + 1) * QW],
                            psb[:],
                        )

    nc.compile()
    return nc


def get_nc(repeat: int = 1):
    key = ("nc", repeat, os.environ.get("BASS_V2_ABL", ""))
    if key not in _CACHE:
        _CACHE[key] = _build(repeat)
    return _CACHE[key]


def make_in_maps(x, mask, Wq, bq, Wk, bk, Wv):
    """Per-core input dict (core c -> batch c//2, head-group c%2)."""
    import ml_dtypes

    bf = ml_dtypes.bfloat16
    x = np.asarray(x, np.float32)
    mask = np.asarray(mask)
    maskbias = (mask == 0).astype(np.float32) * NEG  # [B, S]
    mb2 = np.clip(
        maskbias.astype(np.float64) * EXA + EXB, -3.0e38, 3.0e38
    ).astype(np.float32)  # masked rows saturate int16 -> 0x8000 -> bf16 -0.0
    in_maps = []
    xTb = [np.ascontiguousarray(x[b].T.astype(bf)) for b in range(B)]
    mbTb = [np.ascontiguousarray(maskbias[b].reshape(KT, PART).T) for b in range(B)]
    mbATb = [
        np.ascontiguousarray((maskbias[b] - 3.0).reshape(KT, PART).T)
        for b in range(B)
    ]
    mb2Tb = [np.ascontiguousarray(mb2[b].reshape(KT, PART).T) for b in range(B)]
    slabs = {}
    for g in range(G):
        sl = slice(g * EG, (g + 1) * EG)
        slabs[g] = (
            np.ascontiguousarray(np.asarray(Wq, np.float32)[sl].T.astype(bf)),
            np.ascontiguousarray(np.asarray(Wk, np.float32)[sl].T.astype(bf)),
            np.ascontiguousarray(np.asarray(Wv, np.float32)[sl].T.astype(bf)),
            np.ascontiguousarray(np.asarray(bq, np.float32)[sl].reshape(MT, PART).T),
            np.ascontiguousarray(np.asarray(bk, np.float32)[sl].reshape(MT, PART).T),
        )
    for c in range(NCORES):
        b, g = c // G, c % G
        wq_t, wk_t, wv_t, bq_t, bk_t = slabs[g]
        in_maps.append(
            {
                "xT": xTb[b],
                "wqT": wq_t,
                "wkT": wk_t,
                "wvT": wv_t,
                "mbT": mbTb[b],
                "mbAT": mbATb[b],
                "mb2T": mb2Tb[b],
                "bqT": bq_t,
                "bkT": bk_t,
            }
        )
    return in_maps


def host_tail(mean_attn, text_array, bv, Wo, bo, W1, b1, W2, b2):
    """Exact tail on [B, E]: out_proj (after the mean), normalize, sub, MLP."""
    out = mean_attn + np.asarray(bv, np.float32)[None, :]
    out = out @ np.asarray(Wo, np.float32).T + np.asarray(bo, np.float32)
    out = out / np.linalg.norm(out, axis=-1, keepdims=True)
    out = out - np.asarray(text_array, np.float32)
    h = np.maximum(out @ np.asarray(W1, np.float32).T + np.asarray(b1, np.float32), 0.0)
    return np.tanh(h @ np.asarray(W2, np.float32).T + np.asarray(b2, np.float32))


def kernel(
    x, mask, text_array, Wq, bq, Wk, bk, Wv, bv, Wo, bo, W1, b1, W2, b2
):
    from concourse.bass_utils import run_bass_kernel_spmd

    nc = get_nc()
    in_maps = make_in_maps(x, mask, Wq, bq, Wk, bk, Wv)
    out = run_bass_kernel_spmd(nc, in_maps, core_ids=list(range(NCORES)))
    mean_attn = np.zeros((B, E), np.float32)
    for c in range(NCORES):
        b, g = c // G, c % G
        pz = np.asarray(out.results[c]["resP"][0], np.float32)  # [HG, 65, S]
        r = np.einsum("hdq,hq->hd", pz[:, 0:64, :], 1.0 / pz[:, 64, :])
        mean_attn[b, g * EG : (g + 1) * EG] = r.reshape(EG) / S
    return host_tail(mean_attn, text_array, bv, Wo, bo, W1, b1, W2, b2).astype(
        np.float32
    )


# revision 23
# speedup vs baseline: 1.1244x; 1.1007x over previous
"""Trainium2 Bass kernel for nn_MultiHeadAttention_446676599023.

Strategy (8 NeuronCores, SPMD, no collectives):
  core c -> batch b = c//2, head-group g = c%2 (heads 8g..8g+7, E-dims 512g..512g+512).

Math: reference computes attn_out = softmax(QK^T/sqrt(D)) @ V per head, projects with
Wo, takes mean over sequence, normalizes, subtracts text_array, then a tiny MLP.
mean_S commutes with the output projection, so each core only needs
  r_h[d] = sum_q softmax_row(q) @ V_h  summed over q   (shape [64] per head)
and the whole Wo/normalize/MLP tail runs on host on a [4,1024] tensor (exact algebra,
negligible FLOPs). Device work per core:
  - x^T and all weights live in SBUF as bf16 (halves DMA, enables fast weight load).
  - Q^T,K^T = (Wq x^T) in [d-part, seq-free] bf16; V = x Wv^T in [seq-part, d-free] bf16
    with a ones column per head (65-stride) so the E@V matmul also emits the softmax
    denominator Z as row 64.
  - scores^T[k,q]: lhsT=K^T slice, rhs=Q^T slice (contraction d=64; even/odd heads on
    PE row-groups 0/64 run concurrently).
  - E = exp(scores/8 + maskbias_k), split across two engines to balance throughput:
    ScalarE table exp for 11/16 key-tiles, DVE Schraudolph exp (int32 affine + bitcast,
    ~2-4% rel err, attenuated to <1e-3 by the normalize/subtract tail) for 5/16.
  - P^T[d,q] (+ Z row) = matmul(lhsT=V_aug[k,65], rhs=E^T[k,q]) accumulated over k.
  - finalize per q-chunk: one DVE copy drains P^T/Z to SBUF bf16, DMA ships it out.
Host does r[d] = sum_q P[d,q]/Z[q] (0.014%% of total FLOPs), then /S and the exact
Wo/normalize/MLP tail.
"""

import math
import os
import sys

import numpy as np

for _p in ("/opt/trn_rl_repo",):
    if _p not in sys.path and os.path.isdir(_p):
        sys.path.append(_p)

B, S, E, H = 4, 2048, 1024, 16
D = E // H            # 64 head dim
G = 2                 # head groups (tensor-parallel factor)
EG = E // G           # 512 dims per group
HG = H // G           # 8 heads per group
NCORES = 8
PART = 128
ET = E // PART        # 8 contraction tiles for projections
KT = S // PART        # 16 key tiles
MT = EG // PART       # 4 m-tiles (= head pairs) per group
NEG = -1.0e30

# Schraudolph fast-exp in fp8-e5m2 bit space: bitcast(int8(x*EXA8 + EXB8))
# ~= exp(x) as e5m2. For raw scores s in [-80, 80] the affine lands in
# [3, 117] -- always positive, never in the NaN encodings (>=124), and the
# masked bias saturates the int8 convert to -128 = -0.0.
EXA8 = 5.770780163555851   # 2^2 / ln(2)
EXB8 = 59.825              # 15 * 4 - 366393/2^21 (min max-rel-err bias)

_CACHE: dict = {}


def _build(repeat: int = 1):
    """Build the Bacc module (one SPMD program, same on all 8 cores)."""
    ablate = set(os.environ.get("BASS_V2_ABL", "").split(","))
    import concourse.bacc as bacc
    import concourse.mybir as mybir
    import concourse.tile as tile
    from contextlib import ExitStack

    f32 = mybir.dt.float32
    bf16 = mybir.dt.bfloat16
    f8 = mybir.dt.float8e4   # V tiles (values ~N(0,1): e4m3 3.4% rms)
    f8e = mybir.dt.float8e5  # e tiles (exp up to e^9.8 needs e5m2 range)
    i8 = mybir.dt.int8
    PM = mybir.MatmulPerfMode
    AF = mybir.ActivationFunctionType
    AX = mybir.AxisListType
    ALU = mybir.AluOpType

    nc = bacc.Bacc("TRN2", target_bir_lowering=False, debug=False)
    xT = nc.dram_tensor("xT", [E, S], bf16, kind="ExternalInput").ap()
    wqT = nc.dram_tensor("wqT", [E, EG], bf16, kind="ExternalInput").ap()
    wkT = nc.dram_tensor("wkT", [E, EG], bf16, kind="ExternalInput").ap()
    wvT = nc.dram_tensor("wvT", [E, EG], bf16, kind="ExternalInput").ap()
    mbT = nc.dram_tensor("mbT", [PART, KT], f32, kind="ExternalInput").ap()
    mbAT = nc.dram_tensor("mbAT", [PART, KT], f32, kind="ExternalInput").ap()
    bqT = nc.dram_tensor("bqT", [PART, MT], f32, kind="ExternalInput").ap()
    bkT = nc.dram_tensor("bkT", [PART, MT], f32, kind="ExternalInput").ap()
    resP = nc.dram_tensor(
        "resP", [repeat, HG, 65, S], bf16, kind="ExternalOutput"
    ).ap()

    QC = 4          # q chunks of 512
    QW = S // QC    # 512

    with tile.TileContext(nc) as tc, ExitStack() as ctx:
        const_p = ctx.enter_context(tc.tile_pool(name="const", bufs=1))
        xt_p = ctx.enter_context(tc.tile_pool(name="xt", bufs=ET))
        wv_p = ctx.enter_context(tc.tile_pool(name="wv", bufs=ET))
        wqk_p = ctx.enter_context(tc.tile_pool(name="wqk", bufs=24))
        qt_p = ctx.enter_context(tc.tile_pool(name="qt", bufs=2))
        kt_p = ctx.enter_context(tc.tile_pool(name="kt", bufs=2))
        v_p = ctx.enter_context(tc.tile_pool(name="v", bufs=KT))
        et_p = ctx.enter_context(tc.tile_pool(name="et", bufs=4))
        ei_p = ctx.enter_context(tc.tile_pool(name="ei", bufs=6))
        psb_p = ctx.enter_context(tc.tile_pool(name="psb", bufs=4))
        # PSUM budget (8 banks): scores 5x[128,512] = 5 (per-head tiles so the
        # two exp engines run concurrently on different heads), pacc 2x[65,512]
        # = 2, projection accumulator 1x[128,512] = 1.
        sc_ps = ctx.enter_context(tc.tile_pool(name="scps", bufs=5, space="PSUM"))
        p_ps = ctx.enter_context(tc.tile_pool(name="pps", bufs=2, space="PSUM"))
        qk_ps = ctx.enter_context(tc.tile_pool(name="qkps", bufs=1, space="PSUM"))

        for rep in range(repeat):
            mb = const_p.tile([PART, KT], f32, tag="mb")
            nc.sync.dma_start(mb[:], mbT[:])
            mb8 = const_p.tile([PART, KT], f32, tag="mb8")
            nc.sync.dma_start(mb8[:], mbAT[:])
            bq = const_p.tile([PART, MT], f32, tag="bq")
            nc.sync.dma_start(bq[:], bqT[:])
            bk = const_p.tile([PART, MT], f32, tag="bk")
            nc.sync.dma_start(bk[:], bkT[:])
            ones_c = const_p.tile([PART, HG], bf16, tag="ones_c")
            nc.vector.memset(ones_c[:], 1.0)


            # ---- emission helpers (program order on each engine queue matters:
            # the PE executes in order, so Q/K projection and V projection are
            # interleaved into the attention stream to fill exp-wait gaps) ----
            v_sb = [None] * (KT // 2)  # fp8 pair tiles [128, 2, HG*66]

            def emit_vproj(ks):
                j, sub = ks // 2, ks % 2
                if sub == 0:
                    v_sb[j] = v_p.tile([PART, 2, HG * 66], f8, tag="v", name=f"vp{j}")
                v3 = v_sb[j][:][:, sub, :].rearrange("p (h c) -> p h c", c=66)
                nc.vector.memset(v3[:, :, 64:66], 1.0)
                ps = qk_ps.tile([PART, EG], f32, tag="qkp", name="vps")
                for i in range(ET):
                    nc.tensor.matmul(
                        ps[:],
                        lhsT=xt[i][:, ks * PART : (ks + 1) * PART],
                        rhs=wv[i][:],
                        start=(i == 0),
                        stop=(i == ET - 1),
                    )
                # DVE cannot narrow f32->fp8; ScalarE can
                nc.scalar.copy(
                    v3[:, :, 0:64],
                    ps[:].rearrange("p (h c) -> p h c", c=64),
                )

            qkmats = {}
            wdma = {}

            def emit_wqk_dma(p):
                tiles = []
                for wT in (wkT, wqT):
                    for i in range(ET):
                        t = wqk_p.tile([PART, PART], bf16, tag="wqk")
                        nc.sync.dma_start(
                            t[:],
                            wT[i * PART : (i + 1) * PART, p * PART : (p + 1) * PART],
                        )
                        tiles.append(t)
                wdma[p] = tiles

            def emit_qkproj_group(p, j):
                # j in 0..7: j//4 selects K (0) / Q (1), j%4 the q-chunk
                if j == 0:
                    qkmats[p] = (
                        kt_p.tile([PART, S], bf16, tag="kt", name=f"kt{p}"),
                        qt_p.tile([PART, S], bf16, tag="qt", name=f"qt{p}"),
                    )
                which, qc2 = j // 4, j % 4
                dst = qkmats[p][which]
                bias = (bk, bq)[which]
                wtiles = wdma[p][which * ET : (which + 1) * ET]
                ps = qk_ps.tile([PART, QW], f32, tag="qkp", name="qkps")
                for i in range(ET):
                    nc.tensor.matmul(
                        ps[:],
                        lhsT=wtiles[i][:],
                        rhs=xt[i][:, qc2 * QW : (qc2 + 1) * QW],
                        start=(i == 0),
                        stop=(i == ET - 1),
                    )
                nc.vector.tensor_scalar_add(
                    dst[:, qc2 * QW : (qc2 + 1) * QW],
                    ps[:],
                    bias[:, p : p + 1],
                )

            # DMA order: small weight tiles first (they gate the first
            # projection groups), then x spread across four engine DGE queues
            # so the 4MB load parallelizes across DMA engines
            emit_wqk_dma(0)
            wv = []
            for i in range(ET):
                t = wv_p.tile([PART, EG], bf16, tag="wv")
                nc.sync.dma_start(t[:], wvT[i * PART : (i + 1) * PART, :])
                wv.append(t)
            xt = []
            for i in range(ET):
                t = xt_p.tile([PART, S], bf16, tag="xt")
                nc.sync.dma_start(t[:], xT[i * PART : (i + 1) * PART, :])
                xt.append(t)
            # ---- prologue: Q/K projection for head-pair 0 interleaved with
            # most of the V projection (all pure PE+DVE work that overlaps the
            # input DMA; psum ping-pong keeps the PE streaming) ----
            for j in range(8):
                emit_qkproj_group(0, j)
                emit_vproj(2 * j)

            # ---- attention over head-pairs; V proj folds into (p0, qc0) and
            # next head-pair's Q/K proj into the remaining q-chunks ----
            for p in range(MT):
                kt_m, qt_m = qkmats.pop(p)
                if p + 1 < MT:
                    emit_wqk_dma(p + 1)
                for qc in range(QC):
                    # filler PE work interleaved into the kt loop (executes
                    # during exp waits): V projection streams through (p0,qc0)
                    # two tiles ahead of its consumer; the next head-pair's Q/K
                    # projection spreads over the remaining q-chunks
                    vproj_pipe = p == 0 and qc == 0
                    if p == 0:
                        spread = {0: [], 1: [0, 1, 2], 2: [3, 4, 5], 3: [6, 7]}
                        fillers = [
                            (emit_qkproj_group, 1, j) for j in spread[qc]
                        ]
                    elif p + 1 < MT:
                        fillers = [
                            (emit_qkproj_group, p + 1, j)
                            for j in range(qc * 2, qc * 2 + 2)
                        ]
                    else:
                        fillers = []
                    pacc = {}
                    for hl in (0, 1):
                        pacc[hl] = p_ps.tile(
                            [65, QW], f32, tag="pp", name=f"pacc{hl}"
                        )
                    sct = [[None, None] for _ in range(KT)]

                    def emit_scores(kt):
                        for hl in (0, 1):
                            scps = sc_ps.tile([PART, QW], f32, tag="sc")
                            r0 = hl * 64
                            nc.tensor.matmul(
                                scps[:],
                                lhsT=kt_m[r0 : r0 + 64, kt * PART : (kt + 1) * PART],
                                rhs=qt_m[r0 : r0 + 64, qc * QW : (qc + 1) * QW],
                            )
                            sct[kt][hl] = scps

                    emit_scores(0)
                    for kt in range(KT):
                        # software pipeline: next kt's scores go to the PE queue
                        # BEFORE this kt's EV so the (in-order) PE never blocks
                        # on the exp result with ready work behind it
                        if kt + 1 < KT:
                            emit_scores(kt + 1)
                        if vproj_pipe and kt % 2 == 0:
                            emit_vproj(2 * (kt // 2) + 1)
                        if fillers and kt in (3, 8, 13):
                            f = fillers.pop(0)
                            f[0](*f[1:])
                        # head A: ScalarE exp -> fp8 pair tile (feeds a
                        # DoubleRow EV every second kt); head B: DVE Schraudolph
                        # bf16 -- the two engines work the same kt concurrently
                        if kt % 2 == 0:
                            ea_pair = et_p.tile([PART, 2, QW], f8e, tag="et")
                        nc.scalar.activation(
                            ea_pair[:][:, kt % 2, :],
                            sct[kt][0][:],
                            AF.Exp,
                            bias=mb[:, kt : kt + 1],
                            scale=1.0 / math.sqrt(D),
                        )
                        if kt % 2 == 0:
                            eb_pair = ei_p.tile([PART, 2, QW], i8, tag="ei")
                        nc.vector.tensor_scalar(
                            eb_pair[:][:, kt % 2, :],
                            sct[kt][1][:],
                            EXA8 / 8.0,
                            mb8[:, kt : kt + 1],
                            ALU.mult,
                            ALU.add,
                        )
                        hA = 66 * (2 * p)
                        hB = 66 * (2 * p + 1)
                        if kt % 2 == 1:
                            nc.tensor.matmul(
                                pacc[0][:],
                                lhsT=v_sb[kt // 2][:][:, 0:2, hA : hA + 65],
                                rhs=ea_pair[:][:, 0:2, :],
                                start=(kt == 1),
                                stop=(kt == KT - 1),
                                perf_mode=PM.DoubleRow,
                            )
                            nc.tensor.matmul(
                                pacc[1][:],
                                lhsT=v_sb[kt // 2][:][:, 0:2, hB : hB + 65],
                                rhs=eb_pair[:].bitcast(f8e),
                                start=(kt == 1),
                                stop=(kt == KT - 1),
                                perf_mode=PM.DoubleRow,
                            )
                    # drain P^T/Z to SBUF bf16 and ship to host, which does the
                    # per-q invZ scaling + reduction (tiny)
                    for hl in (0, 1):
                        psb = psb_p.tile([65, QW], bf16, tag="psb")
                        nc.scalar.copy(psb[:], pacc[hl][:])
                        nc.sync.dma_start(
                            resP[rep, 2 * p + hl, :, qc * QW : (qc + 1) * QW],
                            psb[:],
                        )

    nc.compile()
    return nc


def get_nc(repeat: int = 1):
    key = ("nc", repeat, os.environ.get("BASS_V2_ABL", ""))
    if key not in _CACHE:
        _CACHE[key] = _build(repeat)
    return _CACHE[key]


def make_in_maps(x, mask, Wq, bq, Wk, bk, Wv):
    """Per-core input dict (core c -> batch c//2, head-group c%2)."""
    import ml_dtypes

    bf = ml_dtypes.bfloat16
    x = np.asarray(x, np.float32)
    mask = np.asarray(mask)
    maskbias = (mask == 0).astype(np.float32) * NEG  # [B, S]
    in_maps = []
    xTb = [np.ascontiguousarray(x[b].T.astype(bf)) for b in range(B)]
    mbTb = [np.ascontiguousarray(maskbias[b].reshape(KT, PART).T) for b in range(B)]
    mb8 = np.clip(
        maskbias.astype(np.float64) * EXA8 + EXB8, -3.0e38, 3.0e38
    ).astype(np.float32)
    mbATb = [np.ascontiguousarray(mb8[b].reshape(KT, PART).T) for b in range(B)]
    slabs = {}
    for g in range(G):
        sl = slice(g * EG, (g + 1) * EG)
        slabs[g] = (
            np.ascontiguousarray(np.asarray(Wq, np.float32)[sl].T.astype(bf)),
            np.ascontiguousarray(np.asarray(Wk, np.float32)[sl].T.astype(bf)),
            np.ascontiguousarray(np.asarray(Wv, np.float32)[sl].T.astype(bf)),
            np.ascontiguousarray(np.asarray(bq, np.float32)[sl].reshape(MT, PART).T),
            np.ascontiguousarray(np.asarray(bk, np.float32)[sl].reshape(MT, PART).T),
        )
    for c in range(NCORES):
        b, g = c // G, c % G
        wq_t, wk_t, wv_t, bq_t, bk_t = slabs[g]
        in_maps.append(
            {
                "xT": xTb[b],
                "wqT": wq_t,
                "wkT": wk_t,
                "wvT": wv_t,
                "mbT": mbTb[b],
                "mbAT": mbATb[b],
                "bqT": bq_t,
                "bkT": bk_t,
            }
        )
    return in_maps


def host_tail(mean_attn, text_array, bv, Wo, bo, W1, b1, W2, b2):
    """Exact tail on [B, E]: out_proj (after the mean), normalize, sub, MLP."""
    out = mean_attn + np.asarray(bv, np.float32)[None, :]
    out = out @ np.asarray(Wo, np.float32).T + np.asarray(bo, np.float32)
    out = out / np.linalg.norm(out, axis=-1, keepdims=True)
    out = out - np.asarray(text_array, np.float32)
    h = np.maximum(out @ np.asarray(W1, np.float32).T + np.asarray(b1, np.float32), 0.0)
    return np.tanh(h @ np.asarray(W2, np.float32).T + np.asarray(b2, np.float32))


def kernel(
    x, mask, text_array, Wq, bq, Wk, bk, Wv, bv, Wo, bo, W1, b1, W2, b2
):
    from concourse.bass_utils import run_bass_kernel_spmd

    nc = get_nc()
    in_maps = make_in_maps(x, mask, Wq, bq, Wk, bk, Wv)
    out = run_bass_kernel_spmd(nc, in_maps, core_ids=list(range(NCORES)))
    mean_attn = np.zeros((B, E), np.float32)
    for c in range(NCORES):
        b, g = c // G, c % G
        pz = np.asarray(out.results[c]["resP"][0], np.float32)  # [HG, 65, S]
        r = np.einsum("hdq,hq->hd", pz[:, 0:64, :], 1.0 / pz[:, 64, :])
        mean_attn[b, g * EG : (g + 1) * EG] = r.reshape(EG) / S
    return host_tail(mean_attn, text_array, bv, Wo, bo, W1, b1, W2, b2).astype(
        np.float32
    )
